# revision 11
# baseline (speedup 1.0000x reference)
"""Bass/TRN2 kernel for nn_CCAA_15298673508966 (conv chain + axial attention).

Sharding: 8 cores = 4 batches x 2 row-halves. Conv chain runs on row-halves
(data-parallel, halo recompute); the axial attention needs full images, so the
pair exchanges channel-blocks (pairwise AllGather) and each core runs the
attention for 4 heads (32 channels) over the full image.
"""
import sys

sys.path.insert(0, "/opt/trn_rl_repo")

import numpy as np

import concourse.bass as bass
import concourse.bacc as bacc
import concourse.mybir as mybir
import concourse.tile as tile
from concourse.bass_utils import run_bass_kernel_spmd

F32 = mybir.dt.float32
F32R = mybir.dt.float32r
U32 = mybir.dt.uint32
AX = mybir.AluOpType
ACTF = mybir.ActivationFunctionType

B, C, H, W = 4, 64, 256, 256
HEADS = 8
CH = C // HEADS  # 8 channels per head
N_CORES = 8

PITCH = 272  # padded row pitch for x / x11 / x12
OFF = 8      # data column offset inside the pitch
XROWS = 152   # x slice rows   (abs base-12 .. base+140)
R11 = 148     # x11 rows       (abs base-10 .. base+138)
R12 = 136     # x12 rows       (abs base-4  .. base+132)
ROUT = 128    # output rows per core


def _chunks(total, step):
    out = []
    r = 0
    while r < total:
        out.append((r, min(step, total - r)))
        r += step
    return out


def _bap(ap_like, off, dims):
    """Manual AP: keep the partition dim of `ap_like`, custom free dims."""
    return bass.AP(tensor=ap_like.tensor, offset=ap_like.offset + off,
                   ap=[list(ap_like.ap[0])] + [list(d) for d in dims])


def build(dbg=False):
    nc = bacc.Bacc(trn_type="TRN2", target_bir_lowering=False,
                   num_devices=N_CORES)

    din = lambda n, s, dt=F32: nc.dram_tensor(n, s, dt, kind="ExternalInput")
    dout = lambda n, s, dt=F32: nc.dram_tensor(n, s, dt, kind="ExternalOutput")

    xs = din("xs", [128, XROWS, PITCH], F32R)            # [img*64+ch, row, col]
    w5 = din("w5", [128, 25 * 128], F32R)          # block-diag 5x5 stationaries
    w7 = din("w7", [128, 49 * 128], F32R)          # block-diag 7x7 dil2
    wb1 = din("wb1", [64, 9, 128, 128], F32R)      # dw banded K-part 1
    wb2 = din("wb2", [64, 9, 8, 128], F32R)        # dw banded K-part 2
    wp = din("wp", [128, 128], F32R)               # proj block-diag, col-permuted
    wpf = din("wpf", [32, 64], F32R)               # final proj lhsT (my block)
    b5 = din("b5", [128, 1])
    b7 = din("b7", [128, 1])
    bdwr = din("bdwr", [128, 64])                  # b_dw replicated per partition
    bp = din("bp", [128, 1])                       # permuted proj bias
    m5 = din("m5", [128, R11])
    m7 = din("m7", [128, R12])
    onesc = din("onesc", [128, 1], F32R)           # ones column (norm lhsT)
    onesr = din("onesr", [1, 128])                 # ones row (broadcast lhsT)
    ident = din("ident", [128, 128], F32R)               # PE transpose identity
    sel0 = din("sel0", [1, 1], U32)                # 1 iff s == 0
    sel1 = din("sel1", [1, 1], U32)

    po3 = dout("po3", [64, H * W])                 # proj(out3), (x, y)-major
    po4 = dout("po4", [64, H * W])                 # proj(out4), (y, x)-major
    if dbg:
        d_x11 = dout("d_x11", [128, R11, PITCH], F32R)
        d_x12 = dout("d_x12", [128, R12, PITCH], F32R)
        d_xc = dout("d_xc", [128, ROUT, 256], F32R)
        d_g = dout("d_g", [2, 32, 256, 256], F32R)

    with tile.TileContext(nc) as tc:
        with (
            tc.tile_pool(name="dram", bufs=1, space="DRAM") as dram,
            tc.tile_pool(name="consts", bufs=1) as consts,
        ):
            x11 = dram.tile([128, R11, PITCH], F32R, name="x11")
            x12 = dram.tile([128, R12, PITCH], F32R, name="x12")
            xc = dram.tile([128, ROUT, 256], F32R, name="xc")
            mbuf = dram.tile([2, 32, 128, 256], F32R, name="mbuf")
            gin = dram.tile([2, 32, 128, 256], F32R, name="gin")
            gout = dram.tile([2, 2, 32, 128, 256], F32R, name="gout")
            gbuf = dram.tile([2, 32, 256, 256], F32R, name="gbuf")
            gtbuf = dram.tile([2, 32, 256, 256], F32R, name="gtbuf")
            prj3 = dram.tile([32, 256, 256], F32R, name="prj3")
            prj4 = dram.tile([32, 256, 256], F32R, name="prj4")

            b5s = consts.tile([128, 1], F32, name="b5s")
            b7s = consts.tile([128, 1], F32, name="b7s")
            bdws = consts.tile([128, 64], F32, name="bdws")
            bps = consts.tile([128, 1], F32, name="bps")
            m5s = consts.tile([128, R11], F32, name="m5s")
            m7s = consts.tile([128, R12], F32, name="m7s")
            onc = consts.tile([128, 1], F32R, name="onc")
            onr = consts.tile([1, 128], F32, name="onr")
            idn = consts.tile([128, 128], F32R, name="idn")
            for t, src in ((b5s, b5), (b7s, b7), (bdws, bdwr), (bps, bp),
                           (m5s, m5), (m7s, m7), (onc, onesc), (onr, onesr),
                           (idn, ident)):
                nc.sync.dma_start(t[:], src[:])

            # ---------------- PASS A: 5x5 conv + residual -> x11 ------------
            with (
                tc.tile_pool(name="pa", bufs=3) as pa,
                tc.tile_pool(name="paw", bufs=1) as paw,
                tc.tile_pool(name="psa", bufs=3, space="PSUM") as psa,
            ):
                w5s = paw.tile([128, 25 * 128], F32R, name="w5s")
                nc.sync.dma_start(w5s[:], w5[:])
                zb = paw.tile([128, R11 * 8], F32, name="zb")
                nc.vector.memset(zb[:], 0.0)
                # zero the x-halo border columns of x11 / x12
                nc.sync.dma_start(
                    _bap(x11[:, :, :], 0, [[PITCH, R11], [1, 8]]),
                    zb[:, : R11 * 8].bitcast(F32R))
                nc.sync.dma_start(
                    _bap(x11[:, :, :], 264, [[PITCH, R11], [1, 8]]),
                    zb[:, : R11 * 8].bitcast(F32R))
                nc.sync.dma_start(
                    _bap(x12[:, :, :], 0, [[PITCH, R12], [1, 8]]),
                    zb[:, : R12 * 8].bitcast(F32R))
                nc.sync.dma_start(
                    _bap(x12[:, :, :], 264, [[PITCH, R12], [1, 8]]),
                    zb[:, : R12 * 8].bitcast(F32R))

                for r0, g in _chunks(R11, 16):
                    xin = pa.tile([128, 20 + 4, PITCH], F32R, name="xin",
                                  tag="xin")
                    nc.sync.dma_start(xin[:, : g + 4, :],
                                      xs[:, r0: r0 + g + 4, :])
                    for i in range(g // 2):
                        ps = psa.tile([128, 512], F32, name="psA", tag="psA")
                        for t in range(25):
                            ky, kx = divmod(t, 5)
                            rhs = xin[:, 2 * i + ky: 2 * i + ky + 2,
                                      6 + kx: 262 + kx]
                            nc.tensor.matmul(
                                ps[:], w5s[:, t * 128:(t + 1) * 128],
                                rhs,
                                start=(t == 0), stop=(t == 24))
                        tmp = pa.tile([128, 512], F32R, name="tmpA", tag="tmpA")
                        nc.vector.scalar_tensor_tensor(
                            out=tmp[:], in0=ps[:], scalar=b5s[:],
                            in1=_bap(m5s[:, :], r0 + 2 * i, [[1, 2], [0, 256]]),
                            op0=AX.add, op1=AX.mult)
                        nc.vector.tensor_tensor(
                            tmp[:], tmp[:],
                            xin[:, 2 * i + 2: 2 * i + 4, 8:264], AX.add)
                        nc.sync.dma_start(
                            x11[:, r0 + 2 * i: r0 + 2 * i + 2, 8:264], tmp[:])

            # ---------------- PASS B: 7x7 dil2 conv + residual -> x12 -------
            with (
                tc.tile_pool(name="pb", bufs=3) as pb,
                tc.tile_pool(name="pbw", bufs=1) as pbw,
                tc.tile_pool(name="psb", bufs=3, space="PSUM") as psb,
            ):
                w7s = pbw.tile([128, 49 * 128], F32R, name="w7s")
                nc.sync.dma_start(w7s[:], w7[:])
                for r0, g in _chunks(R12, 16):
                    xin = pb.tile([128, 16 + 12, PITCH], F32R, name="xinB",
                                  tag="xinB")
                    nc.sync.dma_start(xin[:, : g + 12, :],
                                      x11[:, r0: r0 + g + 12, :])
                    for i in range(g // 2):
                        ps = psb.tile([128, 512], F32, name="psB", tag="psB")
                        for t in range(49):
                            ky, kx = divmod(t, 7)
                            kyo, kxo = 2 * ky, 2 * kx
                            rhs = xin[:, 2 * i + kyo: 2 * i + kyo + 2,
                                      2 + kxo: 258 + kxo]
                            nc.tensor.matmul(
                                ps[:], w7s[:, t * 128:(t + 1) * 128],
                                rhs,
                                start=(t == 0), stop=(t == 48))
                        tmp = pb.tile([128, 512], F32R, name="tmpB", tag="tmpB")
                        nc.vector.scalar_tensor_tensor(
                            out=tmp[:], in0=ps[:], scalar=b7s[:],
                            in1=_bap(m7s[:, :], r0 + 2 * i, [[1, 2], [0, 256]]),
                            op0=AX.add, op1=AX.mult)
                        nc.vector.tensor_tensor(
                            tmp[:], tmp[:],
                            xin[:, 2 * i + 6: 2 * i + 8, 8:264], AX.add)
                        nc.sync.dma_start(
                            x12[:, r0 + 2 * i: r0 + 2 * i + 2, 8:264], tmp[:])

            # ------- PASS C1: 9x9 depthwise (banded over rows) -> xc --------
            with (
                tc.tile_pool(name="pc", bufs=3) as pc,
                tc.tile_pool(name="pcw", bufs=2) as pcw,
                tc.tile_pool(name="psc", bufs=3, space="PSUM") as psc,
            ):
                for ch in range(64):
                    wb1s = pcw.tile([128, 9 * 128], F32R, name="wb1s",
                                    tag="wb1s")
                    nc.sync.dma_start(
                        wb1s[:], wb1[ch].rearrange("k a b -> a k b"))
                    wb2s = pcw.tile([8, 9 * 128], F32R, name="wb2s", tag="wb2s")
                    nc.sync.dma_start(
                        wb2s[:], wb2[ch].rearrange("k a b -> a k b"))
                    for img in range(2):
                        p = img * 64 + ch
                        t1 = pc.tile([128, PITCH], F32R, name="t1", tag="t1")
                        nc.sync.dma_start(
                            t1[:], bass.AP(tensor=x12.tensor,
                                           offset=p * R12 * PITCH,
                                           ap=[[PITCH, 128], [1, PITCH]]))
                        t2 = pc.tile([8, PITCH], F32R, name="t2", tag="t2")
                        nc.sync.dma_start(
                            t2[:], bass.AP(tensor=x12.tensor,
                                           offset=(p * R12 + 128) * PITCH,
                                           ap=[[PITCH, 8], [1, PITCH]]))
                        res = pc.tile([128, 256], F32R, name="res", tag="res")
                        nc.sync.dma_start(
                            res[:], bass.AP(tensor=x12.tensor,
                                            offset=ch * R12 * PITCH
                                            + 4 * PITCH + OFF,
                                            ap=[[PITCH, 128], [1, 256]]))
                        ps = psc.tile([128, 256], F32, name="psC", tag="psC")
                        for kx in range(9):
                            nc.tensor.matmul(
                                ps[:], wb1s[:, kx * 128:(kx + 1) * 128],
                                t1[:, 4 + kx: 260 + kx],
                                start=(kx == 0), stop=False)
                        for kx in range(9):
                            nc.tensor.matmul(
                                ps[:], wb2s[:, kx * 128:(kx + 1) * 128],
                                t2[:, 4 + kx: 260 + kx],
                                start=False, stop=(kx == 8))
                        oc = pc.tile([128, 256], F32R, name="oc", tag="oc")
                        nc.vector.scalar_tensor_tensor(
                            out=oc[:], in0=ps[:], scalar=bdws[:, ch: ch + 1],
                            in1=res[:], op0=AX.add, op1=AX.add)
                        nc.sync.dma_start(
                            bass.AP(tensor=xc.tensor, offset=p * ROUT * 256,
                                    ap=[[256, 128], [1, 256]]), oc[:])

            # ------- PASS C2: 1x1 proj (+bias, permuted) -> mbuf/gin --------
            with (
                tc.tile_pool(name="pd", bufs=3) as pd,
                tc.tile_pool(name="pdw", bufs=1) as pdw,
                tc.tile_pool(name="psd", bufs=3, space="PSUM") as psd,
            ):
                wps = pdw.tile([128, 128], F32R, name="wps")
                nc.sync.dma_start(wps[:], wp[:])
                for r0 in range(0, ROUT, 8):
                    xt = pd.tile([128, 8, 256], F32R, name="xt", tag="xt")
                    nc.sync.dma_start(xt[:], xc[:, r0: r0 + 8, :])
                    for i in range(4):
                        ps = psd.tile([128, 512], F32, name="psD", tag="psD")
                        rhs = xt[:, 2 * i: 2 * i + 2, :]
                        nc.tensor.matmul(ps[:], wps[:], rhs,
                                         start=True, stop=True)
                        mg = pd.tile([128, 512], F32R, name="mg", tag="mg")
                        nc.vector.tensor_scalar(
                            out=mg[:], in0=ps[:], scalar1=bps[:],
                            scalar2=None, op0=AX.add)
                        rr = r0 + 2 * i
                        nc.sync.dma_start(mbuf[0, :, rr: rr + 2, :],
                                          mg[0:32, :])
                        nc.sync.dma_start(gin[0, :, rr: rr + 2, :],
                                          mg[32:64, :])
                        nc.sync.dma_start(mbuf[1, :, rr: rr + 2, :],
                                          mg[64:96, :])
                        nc.sync.dma_start(gin[1, :, rr: rr + 2, :],
                                          mg[96:128, :])

            # ---------------- exchange: pairwise AllGather ------------------
            nc.gpsimd.collective_compute(
                "AllGather", AX.bypass,
                replica_groups=[[0, 1], [2, 3], [4, 5], [6, 7]],
                ins=[gin.opt()], outs=[gout.opt()])

            with tc.tile_pool(name="pe", bufs=1) as pe:
                s0t = pe.tile([1, 1], U32, name="s0t")
                s1t = pe.tile([1, 1], U32, name="s1t")
                nc.sync.dma_start(s0t[:], sel0[:])
                nc.sync.dma_start(s1t[:], sel1[:])
                r0g = nc.sync.alloc_register("r0g")
                r1g = nc.sync.alloc_register("r1g")
                nc.sync.reg_load(r0g, s0t[0:1, 0:1])
                nc.sync.reg_load(r1g, s1t[0:1, 0:1])
                c0 = nc.sync.snap(r0g, min_val=0, max_val=1)
                c1 = nc.sync.snap(r1g, min_val=0, max_val=1)
                # gbuf rows half h <- rank h's block (mine if h == s)
                nc.sync.dma_start(gbuf[:, :, 0:128, :], gout[0], cond=c1)
                nc.sync.dma_start(gbuf[:, :, 0:128, :], mbuf[:], cond=c0)
                nc.sync.dma_start(gbuf[:, :, 128:256, :], gout[1], cond=c0)
                nc.sync.dma_start(gbuf[:, :, 128:256, :], mbuf[:], cond=c1)

            # ---------------- GT: per-channel spatial transpose -------------
            with (
                tc.tile_pool(name="pt", bufs=4) as pt,
                tc.tile_pool(name="pst", bufs=4, space="PSUM") as pst,
            ):
                for img in range(2):
                    for ch in range(32):
                        for j in range(2):
                            for k in range(2):
                                ti = pt.tile([128, 128], F32R, name="ti",
                                             tag="ti")
                                nc.sync.dma_start(
                                    ti[:], gbuf[img, ch, j * 128:(j + 1) * 128,
                                                k * 128:(k + 1) * 128])
                                tp = pst.tile([128, 128], F32R, name="tp",
                                              tag="tp")
                                nc.tensor.transpose(tp[:], ti[:], idn[:])
                                to = pt.tile([128, 128], F32R, name="to",
                                             tag="to")
                                nc.scalar.copy(to[:], tp[:])
                                nc.sync.dma_start(
                                    gtbuf[img, ch, k * 128:(k + 1) * 128,
                                          j * 128:(j + 1) * 128], to[:])

            if dbg:
                nc.sync.dma_start(d_x11[:], x11[:, :, :])
                nc.sync.dma_start(d_x12[:], x12[:, :, :])
                nc.sync.dma_start(d_xc[:], xc[:, :, :])
                nc.sync.dma_start(d_g[:], gbuf[:, :, :, :])

            # ---------------- axial attention (4 heads x 2 axes) ------------
            with (
                tc.tile_pool(name="qk", bufs=1) as qkp,
                tc.tile_pool(name="at", bufs=2) as at,
                tc.tile_pool(name="vt", bufs=1) as vtp,
                tc.tile_pool(name="ot", bufs=2) as otp,
                tc.tile_pool(name="nrm", bufs=2) as nrm,
                tc.tile_pool(name="psn", bufs=2, space="PSUM") as psn,
            ):
                for axis in range(2):
                    src_q = gtbuf if axis == 0 else gbuf
                    src_kv = gtbuf if axis == 0 else gbuf
                    img_q = 1 if axis == 0 else 0
                    img_kv = 1 - img_q
                    prj = prj3 if axis == 0 else prj4
                    for hd in range(4):
                        cb = hd * 8
                        qts, kts = [], []
                        for i in range(16):
                            cg, tb = divmod(i, 8)
                            qt = qkp.tile([128, 256], F32R, name="qt",
                                          tag=f"qt{i}")
                            nc.sync.dma_start(
                                qt[:], src_q[img_q, cb + cg * 4: cb + cg * 4
                                             + 4, tb * 32:(tb + 1) * 32, :])
                            kt = qkp.tile([128, 256], F32R, name="kt",
                                          tag=f"kt{i}")
                            nc.sync.dma_start(
                                kt[:], src_kv[img_kv, cb + cg * 4: cb + cg * 4
                                              + 4, tb * 32:(tb + 1) * 32, :])
                            qts.append(qt)
                            kts.append(kt)

                        # token norms: nq/nk = sum_d q^2 (ones-matmul)
                        nq = psn.tile([1, 256], F32, name="nq", tag="pA")
                        nk = psn.tile([1, 256], F32, name="nk", tag="pB")
                        for i in range(16):
                            sq = at.tile([128, 256], F32R, name="sq", tag="sq")
                            nc.vector.tensor_tensor(sq[:], qts[i][:],
                                                    qts[i][:], AX.mult)
                            nc.tensor.matmul(nq[:], onc[:],
                                             sq[:],
                                             start=(i == 0), stop=(i == 15))
                            sk = at.tile([128, 256], F32R, name="sk", tag="sq")
                            nc.vector.tensor_tensor(sk[:], kts[i][:],
                                                    kts[i][:], AX.mult)
                            nc.tensor.matmul(nk[:], onc[:],
                                             sk[:],
                                             start=(i == 0), stop=(i == 15))
                        rnq = nrm.tile([1, 256], F32, name="rnq", tag="rnq")
                        rnk = nrm.tile([1, 256], F32, name="rnk", tag="rnk")
                        for dst, src in ((rnq, nq), (rnk, nk)):
                            nc.scalar.activation(dst[:], src[:], ACTF.Sqrt)
                            nc.vector.tensor_scalar(
                                out=dst[:], in0=dst[:], scalar1=1e-12,
                                scalar2=None, op0=AX.max)
                            nc.vector.reciprocal(dst[:], dst[:])

                        # S = q k^T, scaled by rnq (rows) * rnk (cols)
                        pn_sb = []
                        for hf in range(2):
                            sp = psn.tile([128, 256], F32, name="sp",
                                          tag="pA")
                            for i in range(16):
                                nc.tensor.matmul(
                                    sp[:],
                                    qts[i][:, hf * 128:(hf + 1) * 128]
                                    ,
                                    kts[i][:],
                                    start=(i == 0), stop=(i == 15))
                            rqc = psn.tile([128, 1], F32, name="rqc",
                                           tag="pC")
                            nc.tensor.matmul(
                                rqc[:],
                                rnq[:, hf * 128:(hf + 1) * 128], onr[0:1, 0:1])
                            rqs = nrm.tile([128, 1], F32, name="rqs",
                                           tag="rqs")
                            nc.scalar.copy(rqs[:], rqc[:])
                            brk = psn.tile([128, 256], F32, name="brk",
                                           tag="pC")
                            nc.tensor.matmul(brk[:], onr[:], rnk[:])
                            brks = nrm.tile([128, 256], F32, name="brks",
                                            tag="brks")
                            nc.scalar.copy(brks[:], brk[:])
                            s1 = at.tile([128, 256], F32, name="s1", tag="s1")
                            nc.vector.scalar_tensor_tensor(
                                out=s1[:], in0=sp[:], scalar=rqs[:],
                                in1=brks[:], op0=AX.mult, op1=AX.mult)
                            mx = nrm.tile([128, 1], F32, name="mx", tag="mx")
                            nc.vector.reduce_max(mx[:], s1[:],
                                                 mybir.AxisListType.X)
                            nc.vector.tensor_scalar(
                                out=mx[:], in0=mx[:], scalar1=-1.0,
                                scalar2=None, op0=AX.mult)
                            rs = nrm.tile([128, 1], F32, name="rs", tag="rs")
                            pex = at.tile([128, 256], F32, name="pex",
                                          tag="pex")
                            nc.scalar.activation(pex[:], s1[:], ACTF.Exp,
                                                 bias=mx[:], scale=1.0,
                                                 accum_out=rs[:])
                            nc.vector.reciprocal(rs[:], rs[:])
                            pn = at.tile([128, 256], F32R, name="pn",
                                         tag=f"pn{hf}")
                            nc.vector.tensor_scalar(
                                out=pn[:], in0=pex[:], scalar1=rs[:],
                                scalar2=None, op0=AX.mult)
                            pn_sb.append(pn)

                        # transpose P -> PT[gb][g, t], V -> vtok[gb][g, d]
                        ptb = []
                        for gb in range(2):
                            ptt = otp.tile([128, 256], F32R, name="ptt",
                                           tag=f"ptt{gb}")
                            for hf in range(2):
                                pp = psn.tile([128, 128], F32R, name="pp",
                                              tag="pB")
                                nc.tensor.transpose(
                                    pp[:],
                                    pn_sb[hf][:, gb * 128:(gb + 1) * 128],
                                    idn[:])
                                nc.scalar.copy(
                                    ptt[:, hf * 128:(hf + 1) * 128], pp[:])
                            ptb.append(ptt)
                        vtk = []
                        for gb in range(2):
                            vt = vtp.tile([128, 2048], F32R, name="vt",
                                          tag=f"vt{gb}")
                            for i in range(16):
                                vp = psn.tile([128, 128], F32R, name="vp",
                                              tag="pB")
                                nc.tensor.transpose(
                                    vp[:], kts[i][:, gb * 128:(gb + 1) * 128],
                                    idn[:])
                                nc.scalar.copy(
                                    vt[:, i * 128:(i + 1) * 128], vp[:])
                            vtk.append(vt)

                        # OT[d, t] = V^T P^T (+ q * rnq residual)
                        brq = psn.tile([128, 256], F32, name="brq", tag="pB")
                        nc.tensor.matmul(brq[:], onr[:], rnq[:])
                        brqs = nrm.tile([128, 256], F32, name="brqs",
                                        tag="brqs")
                        nc.scalar.copy(brqs[:], brq[:])
                        for i in range(16):
                            op = psn.tile([128, 256], F32, name="op",
                                          tag="pA")
                            for gb in range(2):
                                nc.tensor.matmul(
                                    op[:],
                                    vtk[gb][:, i * 128:(i + 1) * 128]
                                    ,
                                    ptb[gb][:],
                                    start=(gb == 0), stop=(gb == 1))
                            qn = at.tile([128, 256], F32, name="qn", tag="qn")
                            nc.vector.tensor_tensor(qn[:], qts[i][:],
                                                    brqs[:], AX.mult)
                            ot = otp.tile([128, 256], F32R, name="ot", tag="ot")
                            nc.vector.tensor_tensor(ot[:], qn[:], op[:],
                                                    AX.add)
                            cg, tb = divmod(i, 8)
                            nc.sync.dma_start(
                                prj[cb + cg * 4: cb + cg * 4 + 4,
                                    tb * 32:(tb + 1) * 32, :], ot[:])

            # ---------------- final projections ------------------------------
            with (
                tc.tile_pool(name="pf", bufs=3) as pf,
                tc.tile_pool(name="pfw", bufs=1) as pfw,
                tc.tile_pool(name="psf", bufs=3, space="PSUM") as psf,
            ):
                wpfs = pfw.tile([32, 64], F32R, name="wpfs")
                nc.sync.dma_start(wpfs[:], wpf[:])
                for axis in range(2):
                    prj = prj3 if axis == 0 else prj4
                    po = po3 if axis == 0 else po4
                    prf = prj[:, :, :].rearrange("c a b -> c (a b)")
                    for nk_ in range(128):
                        rt = pf.tile([32, 512], F32R, name="rt", tag="rt")
                        nc.sync.dma_start(
                            rt[:], prf[:, nk_ * 512:(nk_ + 1) * 512])
                        ps = psf.tile([64, 512], F32, name="psF", tag="psF")
                        nc.tensor.matmul(ps[:], wpfs[:], rt[:],
                                         start=True, stop=True)
                        ov = pf.tile([64, 512], F32, name="ov", tag="ov")
                        nc.scalar.copy(ov[:], ps[:])
                        nc.sync.dma_start(
                            po[:, nk_ * 512:(nk_ + 1) * 512], ov[:])

    nc.compile()
    return nc


# ---------------------------------------------------------------------------
# host side
# ---------------------------------------------------------------------------

def _host_inputs(x1, x2, w_pt, b_pt, w_d, b_d, w_dw, b_dw, w_proj, b_proj):
    """Build the 8 per-core input maps."""
    f = np.float32
    w5 = np.zeros((128, 25, 128), f)
    wpt = w_pt.reshape(C, C, 25)
    for img in range(2):
        # w5[img*64+ci, t, img*64+co] = w_pt[co, ci, t]
        w5[img * 64:(img + 1) * 64, :, img * 64:(img + 1) * 64] = \
            wpt.transpose(1, 2, 0)
    w7 = np.zeros((128, 49, 128), f)
    wd = w_d.reshape(C, C, 49)
    for img in range(2):
        w7[img * 64:(img + 1) * 64, :, img * 64:(img + 1) * 64] = \
            wd.transpose(1, 2, 0)

    # depthwise banded: out row j (abs base+j) reads x12buf rows j..j+8
    wdw2 = w_dw.reshape(C, 9, 9)
    wb1 = np.zeros((64, 9, 128, 128), f)
    wb2 = np.zeros((64, 9, 8, 128), f)
    for ch in range(64):
        for kx in range(9):
            for j in range(128):
                for ky in range(9):
                    i = j + ky
                    if i < 128:
                        wb1[ch, kx, i, j] = wdw2[ch, ky, kx]
                    else:
                        wb2[ch, kx, i - 128, j] = wdw2[ch, ky, kx]

    ident = np.eye(128, dtype=f)
    onesc = np.ones((128, 1), f)
    onesr = np.ones((1, 128), f)
    b5v = np.tile(b_pt, 2).reshape(128, 1).astype(f)
    b7v = np.tile(b_d, 2).reshape(128, 1).astype(f)
    bdwr = np.tile(b_dw.reshape(1, 64), (128, 1)).astype(f)

    maps = []
    for core in range(N_CORES):
        b, s = divmod(core, 2)
        base = s * 128
        # x slices: rows base-12 .. base+140, cols padded to PITCH at OFF
        xsl = np.zeros((128, XROWS, PITCH), f)
        for img, xi in ((0, x1), (1, x2)):
            lo, hi = base - 12, base + 140
            clo, chi = max(lo, 0), min(hi, H)
            xsl[img * 64:(img + 1) * 64, clo - lo: chi - lo, OFF: OFF + W] = \
                xi[b, :, clo:chi, :]
        # masks: valid rows of x11 / x12
        m5v = np.zeros((128, R11), f)
        for r in range(R11):
            m5v[:, r] = 1.0 if 0 <= base - 10 + r < H else 0.0
        m7v = np.zeros((128, R12), f)
        for r in range(R12):
            m7v[:, r] = 1.0 if 0 <= base - 4 + r < H else 0.0
        # permuted proj: psum col j -> out-ch perm[j]; perm = [mine, partner]
        perm = np.concatenate([np.arange(s * 32, s * 32 + 32),
                               np.arange((1 - s) * 32, (1 - s) * 32 + 32)])
        wpq = w_proj.reshape(C, C)  # [co, ci]
        wpv = np.zeros((128, 128), f)
        for img in range(2):
            wpv[img * 64:(img + 1) * 64, img * 64:(img + 1) * 64] = \
                wpq[perm, :].T
        bpv = np.tile(b_proj[perm], 2).reshape(128, 1).astype(f)
        wpfv = wpq[:, s * 32: s * 32 + 32].T.copy()  # [ci(my), co]

        maps.append({
            "xs": xsl,
            "w5": np.ascontiguousarray(w5.reshape(128, 25 * 128)),
            "w7": np.ascontiguousarray(w7.reshape(128, 49 * 128)),
            "wb1": wb1, "wb2": wb2,
            "wp": wpv, "wpf": wpfv,
            "b5": b5v, "b7": b7v, "bdwr": bdwr, "bp": bpv,
            "m5": m5v, "m7": m7v,
            "onesc": onesc, "onesr": onesr, "ident": ident,
            "sel0": np.array([[1 - s]], np.uint32),
            "sel1": np.array([[s]], np.uint32),
        })
    return maps


_NC_CACHE = {}


def _get_nc(dbg=False):
    if dbg not in _NC_CACHE:
        _NC_CACHE[dbg] = build(dbg)
    return _NC_CACHE[dbg]


def run_cores(dbg=False, trace=False, **inputs):
    args = {k: np.asarray(v, np.float32) for k, v in inputs.items()}
    maps = _host_inputs(**args)
    nc = _get_nc(dbg)
    res = run_bass_kernel_spmd(nc, maps, core_ids=list(range(N_CORES)),
                               trace=trace)
    return res


def assemble(results, x1, x2, b_proj):
    out = np.zeros((B, C, H, W), np.float32)
    for b in range(B):
        r0, r1 = results[2 * b], results[2 * b + 1]
        p3 = (r0["po3"] + r1["po3"]).reshape(C, W, H).transpose(0, 2, 1)
        p4 = (r0["po4"] + r1["po4"]).reshape(C, H, W)
        out[b] = (p3 + p4 + x1[b] + x2[b]
                  + 2.0 * b_proj[:, None, None])
    return out


def kernel(**inputs):
    args = {k: np.asarray(v, np.float32) for k, v in inputs.items()}
    res = run_cores(dbg=False, trace=False, **args)
    return assemble(res.results, args["x1"], args["x2"], args["b_proj"])


if __name__ == "__main__":
    rng = np.random.default_rng(0)
    pass


# revision 12
# speedup vs baseline: 569.1020x; 569.1020x over previous
"""Bass/TRN2 kernel for nn_CCAA_15298673508966 (conv chain + axial attention).

Sharding: 8 cores = 4 batches x 2 row-halves. Conv chain runs on row-halves
(data-parallel, halo recompute); the axial attention needs full images, so the
pair exchanges channel-blocks (pairwise AllGather) and each core runs the
attention for 4 heads (32 channels) over the full image.
"""
import sys

sys.path.insert(0, "/opt/trn_rl_repo")

import numpy as np

import concourse.bass as bass
import concourse.bacc as bacc
import concourse.mybir as mybir
import concourse.tile as tile
from concourse.bass_utils import run_bass_kernel_spmd

F32 = mybir.dt.float32
F32R = mybir.dt.float32r
U32 = mybir.dt.uint32
AX = mybir.AluOpType
ACTF = mybir.ActivationFunctionType

B, C, H, W = 4, 64, 256, 256
HEADS = 8
CH = C // HEADS  # 8 channels per head
N_CORES = 8

PITCH = 272  # padded row pitch for x / x11 / x12
OFF = 8      # data column offset inside the pitch
XROWS = 152   # x slice rows   (abs base-12 .. base+140)
R11 = 148     # x11 rows       (abs base-10 .. base+138)
R12 = 136     # x12 rows       (abs base-4  .. base+132)
ROUT = 128    # output rows per core


def _chunks(total, step):
    out = []
    r = 0
    while r < total:
        out.append((r, min(step, total - r)))
        r += step
    return out


def _bap(ap_like, off, dims):
    """Manual AP: keep the partition dim of `ap_like`, custom free dims."""
    return bass.AP(tensor=ap_like.tensor, offset=ap_like.offset + off,
                   ap=[list(ap_like.ap[0])] + [list(d) for d in dims])


def build(dbg=False):
    nc = bacc.Bacc(trn_type="TRN2", target_bir_lowering=False,
                   num_devices=N_CORES)

    din = lambda n, s, dt=F32: nc.dram_tensor(n, s, dt, kind="ExternalInput")
    dout = lambda n, s, dt=F32: nc.dram_tensor(n, s, dt, kind="ExternalOutput")

    xs = din("xs", [128, XROWS, PITCH], F32R)            # [img*64+ch, row, col]
    w5 = din("w5", [128, 25 * 128], F32R)          # block-diag 5x5 stationaries
    w7 = din("w7", [128, 49 * 128], F32R)          # block-diag 7x7 dil2
    wb1 = din("wb1", [64, 9, 128, 128], F32R)      # dw banded K-part 1
    wb2 = din("wb2", [64, 9, 8, 128], F32R)        # dw banded K-part 2
    wp = din("wp", [128, 128], F32R)               # proj block-diag, col-permuted
    wpf = din("wpf", [32, 64], F32R)               # final proj lhsT (my block)
    b5 = din("b5", [128, 1])
    b7 = din("b7", [128, 1])
    bdwr = din("bdwr", [128, 64])                  # b_dw replicated per partition
    bp = din("bp", [128, 1])                       # permuted proj bias
    m5 = din("m5", [128, R11])
    m7 = din("m7", [128, R12])
    onesc = din("onesc", [128, 1], F32R)           # ones column (norm lhsT)
    onesr = din("onesr", [1, 128])                 # ones row (broadcast lhsT)
    ident = din("ident", [128, 128], F32R)               # PE transpose identity
    sel0 = din("sel0", [1, 1], U32)                # 1 iff s == 0
    sel1 = din("sel1", [1, 1], U32)

    po3 = dout("po3", [64, H * W])                 # proj(out3), (x, y)-major
    po4 = dout("po4", [64, H * W])                 # proj(out4), (y, x)-major
    if dbg:
        d_x11 = dout("d_x11", [128, R11, PITCH], F32R)
        d_x12 = dout("d_x12", [128, R12, PITCH], F32R)
        d_xc = dout("d_xc", [128, ROUT, 256], F32R)
        d_g = dout("d_g", [2, 32, 256, 256], F32R)

    with tile.TileContext(nc) as tc:
        with (
            tc.tile_pool(name="dram", bufs=1, space="DRAM") as dram,
            tc.tile_pool(name="consts", bufs=1) as consts,
        ):
            x11 = dram.tile([128, R11, PITCH], F32R, name="x11")
            x12 = dram.tile([128, R12, PITCH], F32R, name="x12")
            xc = dram.tile([128, ROUT, 256], F32R, name="xc")
            mbuf = dram.tile([2, 32, 128, 256], F32R, name="mbuf")
            gin = dram.tile([2, 32, 128, 256], F32R, name="gin")
            gout = dram.tile([2, 2, 32, 128, 256], F32R, name="gout")
            gbuf = dram.tile([2, 32, 256, 256], F32R, name="gbuf")
            gtbuf = dram.tile([2, 32, 256, 256], F32R, name="gtbuf")
            prj3 = dram.tile([32, 256, 256], F32R, name="prj3")
            prj4 = dram.tile([32, 256, 256], F32R, name="prj4")

            b5s = consts.tile([128, 1], F32, name="b5s")
            b7s = consts.tile([128, 1], F32, name="b7s")
            bdws = consts.tile([128, 64], F32, name="bdws")
            bps = consts.tile([128, 1], F32, name="bps")
            m5s = consts.tile([128, R11], F32, name="m5s")
            m7s = consts.tile([128, R12], F32, name="m7s")
            onc = consts.tile([128, 1], F32R, name="onc")
            onr = consts.tile([1, 128], F32, name="onr")
            idn = consts.tile([128, 128], F32R, name="idn")
            for t, src in ((b5s, b5), (b7s, b7), (bdws, bdwr), (bps, bp),
                           (m5s, m5), (m7s, m7), (onc, onesc), (onr, onesr),
                           (idn, ident)):
                nc.sync.dma_start(t[:], src[:])

            # ---------------- PASS A: 5x5 conv + residual -> x11 ------------
            with (
                tc.tile_pool(name="pa", bufs=3) as pa,
                tc.tile_pool(name="paw", bufs=1) as paw,
                tc.tile_pool(name="psa", bufs=3, space="PSUM") as psa,
            ):
                w5s = paw.tile([128, 25 * 128], F32R, name="w5s")
                nc.sync.dma_start(w5s[:], w5[:])
                zb = paw.tile([128, R11 * 8], F32, name="zb")
                nc.vector.memset(zb[:], 0.0)
                # zero the x-halo border columns of x11 / x12
                nc.sync.dma_start(
                    _bap(x11[:, :, :], 0, [[PITCH, R11], [1, 8]]),
                    zb[:, : R11 * 8].bitcast(F32R))
                nc.sync.dma_start(
                    _bap(x11[:, :, :], 264, [[PITCH, R11], [1, 8]]),
                    zb[:, : R11 * 8].bitcast(F32R))
                nc.sync.dma_start(
                    _bap(x12[:, :, :], 0, [[PITCH, R12], [1, 8]]),
                    zb[:, : R12 * 8].bitcast(F32R))
                nc.sync.dma_start(
                    _bap(x12[:, :, :], 264, [[PITCH, R12], [1, 8]]),
                    zb[:, : R12 * 8].bitcast(F32R))

                for r0, g in _chunks(R11, 16):
                    xin = pa.tile([128, 20 + 4, PITCH], F32R, name="xin",
                                  tag="xin")
                    nc.sync.dma_start(xin[:, : g + 4, :],
                                      xs[:, r0: r0 + g + 4, :])
                    for i in range(g // 2):
                        ps = psa.tile([128, 512], F32, name="psA", tag="psA")
                        for t in range(25):
                            ky, kx = divmod(t, 5)
                            rhs = xin[:, 2 * i + ky: 2 * i + ky + 2,
                                      6 + kx: 262 + kx]
                            nc.tensor.matmul(
                                ps[:], w5s[:, t * 128:(t + 1) * 128],
                                rhs,
                                start=(t == 0), stop=(t == 24))
                        tmp = pa.tile([128, 512], F32R, name="tmpA", tag="tmpA")
                        nc.vector.scalar_tensor_tensor(
                            out=tmp[:], in0=ps[:], scalar=b5s[:],
                            in1=_bap(m5s[:, :], r0 + 2 * i, [[1, 2], [0, 256]]),
                            op0=AX.add, op1=AX.mult)
                        nc.vector.tensor_tensor(
                            tmp[:], tmp[:],
                            xin[:, 2 * i + 2: 2 * i + 4, 8:264], AX.add)
                        nc.sync.dma_start(
                            x11[:, r0 + 2 * i: r0 + 2 * i + 2, 8:264], tmp[:])

            # ---------------- PASS B: 7x7 dil2 conv + residual -> x12 -------
            with (
                tc.tile_pool(name="pb", bufs=3) as pb,
                tc.tile_pool(name="pbw", bufs=1) as pbw,
                tc.tile_pool(name="psb", bufs=3, space="PSUM") as psb,
            ):
                w7s = pbw.tile([128, 49 * 128], F32R, name="w7s")
                nc.sync.dma_start(w7s[:], w7[:])
                for r0, g in _chunks(R12, 16):
                    xin = pb.tile([128, 16 + 12, PITCH], F32R, name="xinB",
                                  tag="xinB")
                    nc.sync.dma_start(xin[:, : g + 12, :],
                                      x11[:, r0: r0 + g + 12, :])
                    for i in range(g // 2):
                        ps = psb.tile([128, 512], F32, name="psB", tag="psB")
                        for t in range(49):
                            ky, kx = divmod(t, 7)
                            kyo, kxo = 2 * ky, 2 * kx
                            rhs = xin[:, 2 * i + kyo: 2 * i + kyo + 2,
                                      2 + kxo: 258 + kxo]
                            nc.tensor.matmul(
                                ps[:], w7s[:, t * 128:(t + 1) * 128],
                                rhs,
                                start=(t == 0), stop=(t == 48))
                        tmp = pb.tile([128, 512], F32R, name="tmpB", tag="tmpB")
                        nc.vector.scalar_tensor_tensor(
                            out=tmp[:], in0=ps[:], scalar=b7s[:],
                            in1=_bap(m7s[:, :], r0 + 2 * i, [[1, 2], [0, 256]]),
                            op0=AX.add, op1=AX.mult)
                        nc.vector.tensor_tensor(
                            tmp[:], tmp[:],
                            xin[:, 2 * i + 6: 2 * i + 8, 8:264], AX.add)
                        nc.sync.dma_start(
                            x12[:, r0 + 2 * i: r0 + 2 * i + 2, 8:264], tmp[:])

            # ------- PASS C1: 9x9 depthwise (banded over rows) -> xc --------
            with (
                tc.tile_pool(name="pc", bufs=3) as pc,
                tc.tile_pool(name="pcw", bufs=2) as pcw,
                tc.tile_pool(name="psc", bufs=3, space="PSUM") as psc,
            ):
                for ch in range(64):
                    wb1s = pcw.tile([128, 9 * 128], F32R, name="wb1s",
                                    tag="wb1s")
                    nc.sync.dma_start(
                        wb1s[:], wb1[ch].rearrange("k a b -> a k b"))
                    wb2s = pcw.tile([8, 9 * 128], F32R, name="wb2s", tag="wb2s")
                    nc.sync.dma_start(
                        wb2s[:], wb2[ch].rearrange("k a b -> a k b"))
                    for img in range(2):
                        p = img * 64 + ch
                        t1 = pc.tile([128, PITCH], F32R, name="t1", tag="t1")
                        nc.sync.dma_start(
                            t1[:], bass.AP(tensor=x12.tensor,
                                           offset=p * R12 * PITCH,
                                           ap=[[PITCH, 128], [1, PITCH]]))
                        t2 = pc.tile([8, PITCH], F32R, name="t2", tag="t2")
                        nc.sync.dma_start(
                            t2[:], bass.AP(tensor=x12.tensor,
                                           offset=(p * R12 + 128) * PITCH,
                                           ap=[[PITCH, 8], [1, PITCH]]))
                        res = pc.tile([128, 256], F32R, name="res", tag="res")
                        nc.sync.dma_start(
                            res[:], bass.AP(tensor=x12.tensor,
                                            offset=ch * R12 * PITCH
                                            + 4 * PITCH + OFF,
                                            ap=[[PITCH, 128], [1, 256]]))
                        ps = psc.tile([128, 256], F32, name="psC", tag="psC")
                        for kx in range(9):
                            nc.tensor.matmul(
                                ps[:], wb1s[:, kx * 128:(kx + 1) * 128],
                                t1[:, 4 + kx: 260 + kx],
                                start=(kx == 0), stop=False)
                        for kx in range(9):
                            nc.tensor.matmul(
                                ps[:], wb2s[:, kx * 128:(kx + 1) * 128],
                                t2[:, 4 + kx: 260 + kx],
                                start=False, stop=(kx == 8))
                        oc = pc.tile([128, 256], F32R, name="oc", tag="oc")
                        nc.vector.scalar_tensor_tensor(
                            out=oc[:], in0=ps[:], scalar=bdws[:, ch: ch + 1],
                            in1=res[:], op0=AX.add, op1=AX.add)
                        nc.sync.dma_start(
                            bass.AP(tensor=xc.tensor, offset=p * ROUT * 256,
                                    ap=[[256, 128], [1, 256]]), oc[:])

            # ------- PASS C2: 1x1 proj (+bias, permuted) -> mbuf/gin --------
            with (
                tc.tile_pool(name="pd", bufs=3) as pd,
                tc.tile_pool(name="pdw", bufs=1) as pdw,
                tc.tile_pool(name="psd", bufs=3, space="PSUM") as psd,
            ):
                wps = pdw.tile([128, 128], F32R, name="wps")
                nc.sync.dma_start(wps[:], wp[:])
                for r0 in range(0, ROUT, 8):
                    xt = pd.tile([128, 8, 256], F32R, name="xt", tag="xt")
                    nc.sync.dma_start(xt[:], xc[:, r0: r0 + 8, :])
                    for i in range(4):
                        ps = psd.tile([128, 512], F32, name="psD", tag="psD")
                        rhs = xt[:, 2 * i: 2 * i + 2, :]
                        nc.tensor.matmul(ps[:], wps[:], rhs,
                                         start=True, stop=True)
                        mg = pd.tile([128, 512], F32R, name="mg", tag="mg")
                        nc.vector.tensor_scalar(
                            out=mg[:], in0=ps[:], scalar1=bps[:],
                            scalar2=None, op0=AX.add)
                        rr = r0 + 2 * i
                        nc.sync.dma_start(mbuf[0, :, rr: rr + 2, :],
                                          mg[0:32, :])
                        nc.sync.dma_start(gin[0, :, rr: rr + 2, :],
                                          mg[32:64, :])
                        nc.sync.dma_start(mbuf[1, :, rr: rr + 2, :],
                                          mg[64:96, :])
                        nc.sync.dma_start(gin[1, :, rr: rr + 2, :],
                                          mg[96:128, :])

            # ---------------- exchange: pairwise AllGather ------------------
            nc.gpsimd.collective_compute(
                "AllGather", AX.bypass,
                replica_groups=[[0, 1], [2, 3], [4, 5], [6, 7]],
                ins=[gin.opt()], outs=[gout.opt()])

            with tc.tile_pool(name="pe", bufs=1) as pe:
                s0t = pe.tile([1, 1], U32, name="s0t")
                s1t = pe.tile([1, 1], U32, name="s1t")
                nc.sync.dma_start(s0t[:], sel0[:])
                nc.sync.dma_start(s1t[:], sel1[:])
                r0g = nc.sync.alloc_register("r0g")
                r1g = nc.sync.alloc_register("r1g")
                nc.sync.reg_load(r0g, s0t[0:1, 0:1])
                nc.sync.reg_load(r1g, s1t[0:1, 0:1])
                c0 = nc.sync.snap(r0g, min_val=0, max_val=1)
                c1 = nc.sync.snap(r1g, min_val=0, max_val=1)
                # gbuf rows half h <- rank h's block (mine if h == s)
                nc.sync.dma_start(gbuf[:, :, 0:128, :], gout[0], cond=c1)
                nc.sync.dma_start(gbuf[:, :, 0:128, :], mbuf[:], cond=c0)
                nc.sync.dma_start(gbuf[:, :, 128:256, :], gout[1], cond=c0)
                nc.sync.dma_start(gbuf[:, :, 128:256, :], mbuf[:], cond=c1)

            # ---------------- GT: per-channel spatial transpose -------------
            with (
                tc.tile_pool(name="pt", bufs=4) as pt,
                tc.tile_pool(name="pst", bufs=4, space="PSUM") as pst,
            ):
                for img in range(2):
                    for ch in range(32):
                        for j in range(2):
                            for k in range(2):
                                ti = pt.tile([128, 128], F32R, name="ti",
                                             tag="ti")
                                nc.sync.dma_start(
                                    ti[:], gbuf[img, ch, j * 128:(j + 1) * 128,
                                                k * 128:(k + 1) * 128])
                                tp = pst.tile([128, 128], F32R, name="tp",
                                              tag="tp")
                                nc.tensor.transpose(tp[:], ti[:], idn[:])
                                to = pt.tile([128, 128], F32R, name="to",
                                             tag="to")
                                nc.scalar.copy(to[:], tp[:])
                                nc.sync.dma_start(
                                    gtbuf[img, ch, k * 128:(k + 1) * 128,
                                          j * 128:(j + 1) * 128], to[:])

            if dbg:
                nc.sync.dma_start(d_x11[:], x11[:, :, :])
                nc.sync.dma_start(d_x12[:], x12[:, :, :])
                nc.sync.dma_start(d_xc[:], xc[:, :, :])
                nc.sync.dma_start(d_g[:], gbuf[:, :, :, :])

            # ---------------- axial attention (4 heads x 2 axes) ------------
            with (
                tc.tile_pool(name="qk", bufs=1) as qkp,
                tc.tile_pool(name="at", bufs=2) as at,
                tc.tile_pool(name="vt", bufs=1) as vtp,
                tc.tile_pool(name="ot", bufs=2) as otp,
                tc.tile_pool(name="nrm", bufs=2) as nrm,
                tc.tile_pool(name="psn", bufs=2, space="PSUM") as psn,
            ):
                for axis in range(2):
                    src_q = gtbuf if axis == 0 else gbuf
                    src_kv = gtbuf if axis == 0 else gbuf
                    img_q = 1 if axis == 0 else 0
                    img_kv = 1 - img_q
                    prj = prj3 if axis == 0 else prj4
                    for hd in range(4):
                        cb = hd * 8
                        qts, kts = [], []
                        for i in range(16):
                            cg, tb = divmod(i, 8)
                            qt = qkp.tile([128, 256], F32R, name="qt",
                                          tag=f"qt{i}")
                            nc.sync.dma_start(
                                qt[:], src_q[img_q, cb + cg * 4: cb + cg * 4
                                             + 4, tb * 32:(tb + 1) * 32, :])
                            kt = qkp.tile([128, 256], F32R, name="kt",
                                          tag=f"kt{i}")
                            nc.sync.dma_start(
                                kt[:], src_kv[img_kv, cb + cg * 4: cb + cg * 4
                                              + 4, tb * 32:(tb + 1) * 32, :])
                            qts.append(qt)
                            kts.append(kt)

                        # token norms: nq/nk = sum_d q^2 (ones-matmul)
                        nq = psn.tile([1, 256], F32, name="nq", tag="pA")
                        nk = psn.tile([1, 256], F32, name="nk", tag="pB")
                        for i in range(16):
                            sq = at.tile([128, 256], F32R, name="sq", tag="sq")
                            nc.vector.tensor_tensor(sq[:], qts[i][:],
                                                    qts[i][:], AX.mult)
                            nc.tensor.matmul(nq[:], onc[:],
                                             sq[:],
                                             start=(i == 0), stop=(i == 15))
                            sk = at.tile([128, 256], F32R, name="sk", tag="sq")
                            nc.vector.tensor_tensor(sk[:], kts[i][:],
                                                    kts[i][:], AX.mult)
                            nc.tensor.matmul(nk[:], onc[:],
                                             sk[:],
                                             start=(i == 0), stop=(i == 15))
                        rnq = nrm.tile([1, 256], F32, name="rnq", tag="rnq")
                        rnk = nrm.tile([1, 256], F32, name="rnk", tag="rnk")
                        for dst, src in ((rnq, nq), (rnk, nk)):
                            nc.scalar.activation(dst[:], src[:], ACTF.Sqrt)
                            nc.vector.tensor_scalar(
                                out=dst[:], in0=dst[:], scalar1=1e-12,
                                scalar2=None, op0=AX.max)
                            nc.vector.reciprocal(dst[:], dst[:])

                        # S = q k^T, scaled by rnq (rows) * rnk (cols)
                        pn_sb = []
                        for hf in range(2):
                            sp = psn.tile([128, 256], F32, name="sp",
                                          tag="pA")
                            for i in range(16):
                                nc.tensor.matmul(
                                    sp[:],
                                    qts[i][:, hf * 128:(hf + 1) * 128]
                                    ,
                                    kts[i][:],
                                    start=(i == 0), stop=(i == 15))
                            rqc = psn.tile([128, 1], F32, name="rqc",
                                           tag="pC")
                            nc.tensor.matmul(
                                rqc[:],
                                rnq[:, hf * 128:(hf + 1) * 128], onr[0:1, 0:1])
                            rqs = nrm.tile([128, 1], F32, name="rqs",
                                           tag="rqs")
                            nc.scalar.copy(rqs[:], rqc[:])
                            brk = psn.tile([128, 256], F32, name="brk",
                                           tag="pC")
                            nc.tensor.matmul(brk[:], onr[:], rnk[:])
                            brks = nrm.tile([128, 256], F32, name="brks",
                                            tag="brks")
                            nc.scalar.copy(brks[:], brk[:])
                            s1 = at.tile([128, 256], F32, name="s1", tag="s1")
                            nc.vector.scalar_tensor_tensor(
                                out=s1[:], in0=sp[:], scalar=rqs[:],
                                in1=brks[:], op0=AX.mult, op1=AX.mult)
                            mx = nrm.tile([128, 1], F32, name="mx", tag="mx")
                            nc.vector.reduce_max(mx[:], s1[:],
                                                 mybir.AxisListType.X)
                            nc.vector.tensor_scalar(
                                out=mx[:], in0=mx[:], scalar1=-1.0,
                                scalar2=None, op0=AX.mult)
                            rs = nrm.tile([128, 1], F32, name="rs", tag="rs")
                            pex = at.tile([128, 256], F32, name="pex",
                                          tag="pex")
                            nc.scalar.activation(pex[:], s1[:], ACTF.Exp,
                                                 bias=mx[:], scale=1.0,
                                                 accum_out=rs[:])
                            nc.vector.reciprocal(rs[:], rs[:])
                            pn = at.tile([128, 256], F32R, name="pn",
                                         tag=f"pn{hf}")
                            nc.vector.tensor_scalar(
                                out=pn[:], in0=pex[:], scalar1=rs[:],
                                scalar2=None, op0=AX.mult)
                            pn_sb.append(pn)

                        # transpose P -> PT[gb][g, t], V -> vtok[gb][g, d]
                        ptb = []
                        for gb in range(2):
                            ptt = otp.tile([128, 256], F32R, name="ptt",
                                           tag=f"ptt{gb}")
                            for hf in range(2):
                                pp = psn.tile([128, 128], F32R, name="pp",
                                              tag="pB")
                                nc.tensor.transpose(
                                    pp[:],
                                    pn_sb[hf][:, gb * 128:(gb + 1) * 128],
                                    idn[:])
                                nc.scalar.copy(
                                    ptt[:, hf * 128:(hf + 1) * 128], pp[:])
                            ptb.append(ptt)
                        vtk = []
                        for gb in range(2):
                            vt = vtp.tile([128, 2048], F32R, name="vt",
                                          tag=f"vt{gb}")
                            for i in range(16):
                                vp = psn.tile([128, 128], F32R, name="vp",
                                              tag="pB")
                                nc.tensor.transpose(
                                    vp[:], kts[i][:, gb * 128:(gb + 1) * 128],
                                    idn[:])
                                nc.scalar.copy(
                                    vt[:, i * 128:(i + 1) * 128], vp[:])
                            vtk.append(vt)

                        # OT[d, t] = V^T P^T (+ q * rnq residual)
                        brq = psn.tile([128, 256], F32, name="brq", tag="pB")
                        nc.tensor.matmul(brq[:], onr[:], rnq[:])
                        brqs = nrm.tile([128, 256], F32, name="brqs",
                                        tag="brqs")
                        nc.scalar.copy(brqs[:], brq[:])
                        for i in range(16):
                            op = psn.tile([128, 256], F32, name="op",
                                          tag="pA")
                            for gb in range(2):
                                nc.tensor.matmul(
                                    op[:],
                                    vtk[gb][:, i * 128:(i + 1) * 128]
                                    ,
                                    ptb[gb][:],
                                    start=(gb == 0), stop=(gb == 1))
                            qn = at.tile([128, 256], F32, name="qn", tag="qn")
                            nc.vector.tensor_tensor(qn[:], qts[i][:],
                                                    brqs[:], AX.mult)
                            ot = otp.tile([128, 256], F32R, name="ot", tag="ot")
                            nc.vector.tensor_tensor(ot[:], qn[:], op[:],
                                                    AX.add)
                            cg, tb = divmod(i, 8)
                            nc.sync.dma_start(
                                prj[cb + cg * 4: cb + cg * 4 + 4,
                                    tb * 32:(tb + 1) * 32, :], ot[:])

            # ---------------- final projections ------------------------------
            with (
                tc.tile_pool(name="pf", bufs=3) as pf,
                tc.tile_pool(name="pfw", bufs=1) as pfw,
                tc.tile_pool(name="psf", bufs=3, space="PSUM") as psf,
            ):
                wpfs = pfw.tile([32, 64], F32R, name="wpfs")
                nc.sync.dma_start(wpfs[:], wpf[:])
                for axis in range(2):
                    prj = prj3 if axis == 0 else prj4
                    po = po3 if axis == 0 else po4
                    prf = prj[:, :, :].rearrange("c a b -> c (a b)")
                    for nk_ in range(128):
                        rt = pf.tile([32, 512], F32R, name="rt", tag="rt")
                        nc.sync.dma_start(
                            rt[:], prf[:, nk_ * 512:(nk_ + 1) * 512])
                        ps = psf.tile([64, 512], F32, name="psF", tag="psF")
                        nc.tensor.matmul(ps[:], wpfs[:], rt[:],
                                         start=True, stop=True)
                        ov = pf.tile([64, 512], F32, name="ov", tag="ov")
                        nc.scalar.copy(ov[:], ps[:])
                        nc.sync.dma_start(
                            po[:, nk_ * 512:(nk_ + 1) * 512], ov[:])

    nc.compile()
    return nc


# ---------------------------------------------------------------------------
# host side
# ---------------------------------------------------------------------------

def _host_inputs(x1, x2, w_pt, b_pt, w_d, b_d, w_dw, b_dw, w_proj, b_proj):
    """Build the 8 per-core input maps."""
    f = np.float32
    w5 = np.zeros((128, 25, 128), f)
    wpt = w_pt.reshape(C, C, 25)
    for img in range(2):
        # w5[img*64+ci, t, img*64+co] = w_pt[co, ci, t]
        w5[img * 64:(img + 1) * 64, :, img * 64:(img + 1) * 64] = \
            wpt.transpose(1, 2, 0)
    w7 = np.zeros((128, 49, 128), f)
    wd = w_d.reshape(C, C, 49)
    for img in range(2):
        w7[img * 64:(img + 1) * 64, :, img * 64:(img + 1) * 64] = \
            wd.transpose(1, 2, 0)

    # depthwise banded: out row j (abs base+j) reads x12buf rows j..j+8
    wdw2 = w_dw.reshape(C, 9, 9).transpose(0, 2, 1)  # [ch, kx, ky]
    wb1 = np.zeros((64, 9, 128, 128), f)
    wb2 = np.zeros((64, 9, 8, 128), f)
    for ky in range(9):
        j1 = np.arange(128 - ky)
        wb1[:, :, j1 + ky, j1] = wdw2[:, :, ky][:, :, None]
        j2 = np.arange(128 - ky, 128)
        if len(j2):
            wb2[:, :, j2 + ky - 128, j2] = wdw2[:, :, ky][:, :, None]

    ident = np.eye(128, dtype=f)
    onesc = np.ones((128, 1), f)
    onesr = np.ones((1, 128), f)
    b5v = np.tile(b_pt, 2).reshape(128, 1).astype(f)
    b7v = np.tile(b_d, 2).reshape(128, 1).astype(f)
    bdwr = np.tile(b_dw.reshape(1, 64), (128, 1)).astype(f)

    maps = []
    for core in range(N_CORES):
        b, s = divmod(core, 2)
        base = s * 128
        # x slices: rows base-12 .. base+140, cols padded to PITCH at OFF
        xsl = np.zeros((128, XROWS, PITCH), f)
        for img, xi in ((0, x1), (1, x2)):
            lo, hi = base - 12, base + 140
            clo, chi = max(lo, 0), min(hi, H)
            xsl[img * 64:(img + 1) * 64, clo - lo: chi - lo, OFF: OFF + W] = \
                xi[b, :, clo:chi, :]
        # masks: valid rows of x11 / x12
        m5v = np.zeros((128, R11), f)
        for r in range(R11):
            m5v[:, r] = 1.0 if 0 <= base - 10 + r < H else 0.0
        m7v = np.zeros((128, R12), f)
        for r in range(R12):
            m7v[:, r] = 1.0 if 0 <= base - 4 + r < H else 0.0
        # permuted proj: psum col j -> out-ch perm[j]; perm = [mine, partner]
        perm = np.concatenate([np.arange(s * 32, s * 32 + 32),
                               np.arange((1 - s) * 32, (1 - s) * 32 + 32)])
        wpq = w_proj.reshape(C, C)  # [co, ci]
        wpv = np.zeros((128, 128), f)
        for img in range(2):
            wpv[img * 64:(img + 1) * 64, img * 64:(img + 1) * 64] = \
                wpq[perm, :].T
        bpv = np.tile(b_proj[perm], 2).reshape(128, 1).astype(f)
        wpfv = wpq[:, s * 32: s * 32 + 32].T.copy()  # [ci(my), co]

        maps.append({
            "xs": xsl,
            "w5": np.ascontiguousarray(w5.reshape(128, 25 * 128)),
            "w7": np.ascontiguousarray(w7.reshape(128, 49 * 128)),
            "wb1": wb1, "wb2": wb2,
            "wp": wpv, "wpf": wpfv,
            "b5": b5v, "b7": b7v, "bdwr": bdwr, "bp": bpv,
            "m5": m5v, "m7": m7v,
            "onesc": onesc, "onesr": onesr, "ident": ident,
            "sel0": np.array([[1 - s]], np.uint32),
            "sel1": np.array([[s]], np.uint32),
        })
    return maps


_NC_CACHE = {}


def _get_nc(dbg=False):
    if dbg not in _NC_CACHE:
        _NC_CACHE[dbg] = build(dbg)
    return _NC_CACHE[dbg]


def run_cores(dbg=False, trace=False, **inputs):
    args = {k: np.asarray(v, np.float32) for k, v in inputs.items()}
    maps = _host_inputs(**args)
    nc = _get_nc(dbg)
    res = run_bass_kernel_spmd(nc, maps, core_ids=list(range(N_CORES)),
                               trace=trace)
    return res


def assemble(results, x1, x2, b_proj):
    out = np.zeros((B, C, H, W), np.float32)
    for b in range(B):
        r0, r1 = results[2 * b], results[2 * b + 1]
        p3 = (r0["po3"] + r1["po3"]).reshape(C, W, H).transpose(0, 2, 1)
        p4 = (r0["po4"] + r1["po4"]).reshape(C, H, W)
        out[b] = (p3 + p4 + x1[b] + x2[b]
                  + 2.0 * b_proj[:, None, None])
    return out


def kernel(**inputs):
    args = {k: np.asarray(v, np.float32) for k, v in inputs.items()}
    res = run_cores(dbg=False, trace=False, **args)
    return assemble(res.results, args["x1"], args["x2"], args["b_proj"])


if __name__ == "__main__":
    rng = np.random.default_rng(0)
    pass
